# revision 41
# baseline (speedup 1.0000x reference)
"""3-layer GCN block (improved gcn_norm, identity activations, residuals)
on 8 Trainium2 NeuronCores.

Strategy (graph/data parallel, dst-sharded) — tight-packed schedule:
  - Nodes are permuted into 784 tiles of 128 (serpentine bin-packing on
    in-degree); 98 tiles per core, processed in 17 groups (16x6 + 1x2).
  - Aggregation commutes with the dense transform:  Ahat(X W) = (Ahat X) W,
    so each core aggregates raw features for its 12544-node shard and then
    applies the 128x128 weight to the shard only.
  - The gather table holds t[u] = bf16(out[u] * dinv[u]); per-edge weights
    factor as norm_e = dinv[dst] * t-scale, so selection matrices are exact
    0/1 one-hots, self-loops contribute 2*t[u] (a 2I matmul fed by one
    contiguous DMA per group), and dinv[dst] folds into the epilogue.
  - Per (group, src-range) block, the 6 tiles' real edges are packed
    back-to-back (tile-major, src-sorted) into 128-edge chunks with NO
    per-tile alignment: a chunk spanning a tile boundary simply gets one
    selection column (and one PSUM matmul) per spanned tile.  Chunk counts
    and chunk->tile target sets are core-uniform by taking max/union over
    the 8 cores; per-core shortfall slots carry dstsel=800 -> SEL 0.
  - Gather calls batch up to NCH_CALL chunks; DVE builds the one-hot
    SEL[e, col] = (dstsel[e, col] == iota[d]) in bf16 (2x DVE mode), and
    the segment-sum becomes PSUM-accumulated bf16 matmuls
    aggT[feat, dst] += msg[e, feat].T @ SEL.
  - Epilogue per group (batched single DMAs): aggT.T @ W (+bias, +residual,
    * dinv[dst]) in fp32; layer-1 pre-adds the global residual x into its
    saved residual so layer 2 needs one fewer stream.
  - Between layers the 8 bf16 shards are exchanged with an AllGather into a
    Shared DRAM buffer that next layer's gathers read.

Everything data-dependent in the instruction stream is core-uniform by
construction so one BIR program runs SPMD on all 8 cores.
"""
import numpy as np

P = 128
D = 128
NCORES = 8
NCH_CALL = 8           # chunks per dma_gather call (1024-descriptor ucode cap)


class _Cfg:
    def __init__(self, n_nodes, tiles_per_core=102, group_t=6, nrange=4):
        self.N = n_nodes
        self.TPC = tiles_per_core
        self.SHARD = tiles_per_core * P
        self.NPAD = NCORES * self.SHARD
        self.NT = NCORES * tiles_per_core
        self.NRANGE = nrange
        assert self.NPAD % nrange == 0
        self.RSZ = self.NPAD // nrange
        assert self.RSZ <= 32768, "int16 gather index range exceeded"
        gs = [group_t] * (tiles_per_core // group_t)
        if tiles_per_core % group_t:
            gs.append(tiles_per_core % group_t)
        self.GS = gs


CFG = _Cfg(100000)
PAD_VAL = 800.0        # dstsel value that matches no iota column (bf16-exact)


def _trow(padded_id, TPC, HT, HROWS):
    """Shard-layout padded id (core-major) -> half-major table row."""
    p = padded_id % P
    t = padded_id // P
    core = t // TPC
    tl = t % TPC
    return np.where(
        tl < HT,
        (core * HT + tl) * P + p,
        HROWS + (core * HT + (tl - HT)) * P + p)


def _host_prep(edge_index, cfg, seed0=0):
    """Permute nodes; build the tight-packed core-uniform chunk schedule."""
    import ml_dtypes

    N, NPAD, NT, TPC = cfg.N, cfg.NPAD, cfg.NT, cfg.TPC
    SHARD, RSZ, NR, GS = cfg.SHARD, cfg.RSZ, cfg.NRANGE, cfg.GS
    NG = len(GS)
    t0_of_g = np.concatenate([[0], np.cumsum(GS)])[:-1]
    g_of_tl = np.repeat(np.arange(NG), GS)

    src = edge_index[0].astype(np.int64)
    dst = edge_index[1].astype(np.int64)

    indeg = np.bincount(dst, minlength=N)
    deg = (indeg + 2).astype(np.float32)
    dinv = (1.0 / np.sqrt(deg)).astype(np.float32)

    # table rows are HALF-MAJOR: [core0 tiles 0:HT | ... | core7 tiles 0:HT |
    # core0 tiles HT: | ...] so the inter-layer AllGather can run as two
    # halves, the first overlapping the producing layer's tail.
    HT = TPC // 2
    HROWS = NCORES * HT * P

    # ---- node -> padded id via serpentine on in-degree; minimize padded
    # slot total of the block schedule ----
    Lall = np.zeros(NPAD, np.float64)
    Lall[:N] = indeg + 1
    best = None
    for attempt in range(10):
        rng = np.random.default_rng(seed0 + attempt)
        order = np.argsort(-(Lall + rng.random(NPAD)), kind="stable")
        ranks = np.empty(NPAD, np.int64)
        ranks[order] = np.arange(NPAD)
        blk, j = ranks // NT, ranks % NT
        tile = np.where(blk % 2 == 0, j, NT - 1 - j)
        cand = tile * P + blk
        t_e = cand[dst] // P
        tl_e = t_e % TPC
        core_e = t_e // TPC
        r_e = _trow(cand[src], TPC, HT, HROWS) // RSZ
        g_e = g_of_tl[tl_e]
        # per (core, g, r) block totals; objective = gather calls, then slots
        bid = (core_e * NG + g_e) * NR + r_e
        B = np.bincount(bid, minlength=NCORES * NG * NR).reshape(NCORES, NG * NR)
        nch_a = np.ceil(B.max(axis=0) / P)
        cost = (int(np.ceil(nch_a / 8).sum()), int(nch_a.sum()))
        if best is None or cost < best[0]:
            best = (cost, cand)
    pid = best[1]

    # pid = core-major shard row (shard-side layout); trow_of = table row
    trow_of = _trow(np.arange(NPAD, dtype=np.int64), TPC, HT, HROWS)

    e_src = pid[src]
    e_dst = pid[dst]
    t_e = e_dst // P
    core_e = t_e // TPC
    tl_e = t_e % TPC
    g_e = g_of_tl[tl_e]
    tlg_e = tl_e - t0_of_g[g_e]
    srow_e = trow_of[e_src]
    r_e = srow_e // RSZ
    srcl_e = srow_e % RSZ
    dstm_e = e_dst % P

    # per (core, tile-in-core, range) counts for segment boundaries
    cnt = np.bincount((core_e * TPC + tl_e) * NR + r_e,
                      minlength=NCORES * TPC * NR).reshape(NCORES, TPC, NR)

    # ---- block schedule (core-uniform) ----
    # B[k, g, r], nch per block, per-core tile segment starts within block
    Bkgr = np.zeros((NCORES, NG, NR), np.int64)
    for g in range(NG):
        tls = slice(t0_of_g[g], t0_of_g[g] + GS[g])
        Bkgr[:, g, :] = cnt[:, tls, :].sum(axis=1)
    nch_b = np.ceil(Bkgr.max(axis=0) / P).astype(np.int64)  # [NG, NR]

    # program order: g outer, r inner
    blocks = []           # (g, r, c0, nch, col0, ncols, colmap, mms)
    CHUNKS = 0
    COLS = 0
    # last column per global tile-in-core for stop flags
    last_col_of_tile = {}
    colmaps = {}
    for g in range(NG):
        T = GS[g]
        for r in range(NR):
            nch = int(nch_b[g, r])
            # per-core segment boundaries within the block
            seg = np.zeros((NCORES, T + 1), np.int64)
            seg[:, 1:] = np.cumsum(cnt[:, t0_of_g[g]:t0_of_g[g] + T, r], axis=1)
            # chunk -> target tiles (union over cores)
            colmap = -np.ones((nch, T), np.int64)
            ncols = 0
            mms = []
            for j in range(nch):
                lo, hi = j * P, (j + 1) * P
                tgts = set()
                for k in range(NCORES):
                    s, e = seg[k, :-1], seg[k, 1:]
                    for tl in np.nonzero((s < hi) & (e > lo))[0]:
                        tgts.add(int(tl))
                for tl in sorted(tgts):
                    colmap[j, tl] = COLS + ncols
                    mms.append([j, tl, False])   # chunk-in-block, tile, stop
                    last_col_of_tile[(g, tl)] = (len(blocks), len(mms) - 1)
                    ncols += 1
            blocks.append(dict(g=g, r=r, c0=CHUNKS, nch=nch,
                               col0=COLS, ncols=ncols, mms=mms))
            colmaps[(g, r)] = colmap
            CHUNKS += nch
            COLS += ncols
    for (g, tl), (bi, mi) in last_col_of_tile.items():
        blocks[bi]["mms"][mi][2] = True

    # ---- per-edge slot assignment ----
    # order: (core, g, r, tl, src) ; slot = rank within (core, g, r)
    key_block = ((core_e * NG + g_e) * NR + r_e)
    okey = np.lexsort((srcl_e, tlg_e, key_block))
    kb_s = key_block[okey]
    bstart = np.zeros(NCORES * NG * NR + 1, np.int64)
    np.cumsum(np.bincount(kb_s, minlength=NCORES * NG * NR), out=bstart[1:])
    slot = np.arange(kb_s.shape[0]) - bstart[kb_s]

    c0_of = np.zeros((NG, NR), np.int64)
    for b in blocks:
        c0_of[b["g"], b["r"]] = b["c0"]

    g_s = g_e[okey]
    r_s = r_e[okey]
    tlg_s = tlg_e[okey]
    core_s = core_e[okey]
    j_s = slot // P
    p_s = slot % P
    C_s = c0_of[g_s, r_s] + j_s       # global chunk
    # column of each edge
    col_s = np.empty(kb_s.shape[0], np.int64)
    for (g, r), cm in colmaps.items():
        m = (g_s == g) & (r_s == r)
        col_s[m] = cm[j_s[m], tlg_s[m]]
    assert col_s.min() >= 0

    dstsel = np.full((NCORES, P, COLS), PAD_VAL, ml_dtypes.bfloat16)
    flat = (core_s * P + p_s) * COLS + col_s
    dstsel.reshape(-1)[flat] = dstm_e[okey].astype(np.float32)

    ICOLS = CHUNKS * 8
    idxs16 = np.zeros((NCORES, 16, ICOLS), np.int16)
    icol = C_s * 8 + p_s // 16
    iflat = (core_s * 16 + (p_s % 16)) * ICOLS + icol
    idxs16.reshape(-1)[iflat] = srcl_e[okey].astype(np.int16)
    idxs16 = np.tile(idxs16, (1, 8, 1))

    # gather calls per block: split nch into pieces <= NCH_CALL, and build
    # per-call matmul lists
    for b in blocks:
        calls = []
        c = 0
        mm_by_chunk = {}
        for (j, tl, stop) in b["mms"]:
            mm_by_chunk.setdefault(j, []).append((tl, stop))
        colc = 0
        while c < b["nch"]:
            nchc = min(NCH_CALL, b["nch"] - c)
            mmc = []
            ncolsc = 0
            for j in range(c, c + nchc):
                for (tl, stop) in mm_by_chunk.get(j, ()):
                    mmc.append((ncolsc, j - c, tl, stop))
                    ncolsc += 1
            calls.append(dict(c0=b["c0"] + c, nch=nchc,
                              col0=b["col0"] + colc, ncols=ncolsc, mms=mmc))
            c += nchc
            colc += ncolsc
        b["calls"] = calls

    dinv_pad = np.zeros(NPAD, np.float32)
    dinv_pad[pid[:N]] = dinv
    dinv_tiles = np.ascontiguousarray(
        dinv_pad.reshape(NCORES, TPC, P).transpose(0, 2, 1))

    sched = dict(blocks=blocks, CHUNKS=CHUNKS, COLS=COLS, ICOLS=ICOLS)
    return dict(
        pid=pid, trow_of=trow_of, dinv_pad=dinv_pad, dinv_tiles=dinv_tiles,
        dstsel=dstsel, idxs16=idxs16, sched=sched, bf16=ml_dtypes.bfloat16,
    )


# ------------------------------------------------------------------ device --

_NC_CACHE = {}


def _build_nc(cfg, sched, nlayers=3):
    skey = []
    for b in sched["blocks"]:
        skey.append((b["g"], b["r"], b["c0"], b["nch"], b["col0"], b["ncols"],
                     tuple(tuple(c["mms"]) for c in b["calls"])))
    key = (cfg.N, cfg.TPC, nlayers, sched["COLS"], sched["CHUNKS"],
           tuple(skey))
    if key in _NC_CACHE:
        return _NC_CACHE[key]

    import concourse.bacc as bacc
    import concourse.mybir as mybir
    import concourse.tile as tile

    NPAD, SHARD, TPC, RSZ, NR, GS = (
        cfg.NPAD, cfg.SHARD, cfg.TPC, cfg.RSZ, cfg.NRANGE, cfg.GS)
    NG = len(GS)
    t0_of_g = np.concatenate([[0], np.cumsum(GS)])[:-1]
    COLS, ICOLS = sched["COLS"], sched["ICOLS"]
    f32 = mybir.dt.float32
    bf16 = mybir.dt.bfloat16

    nc = bacc.Bacc("TRN2", target_bir_lowering=False, debug=False,
                   num_devices=NCORES, num_swdge_queues=4,
                   dynamic_dma_scratch_size=49152)

    tfull0 = nc.dram_tensor("tfull0", [NPAD, D], bf16, kind="ExternalInput")
    tsh0 = nc.dram_tensor("tsh0", [SHARD, D], bf16, kind="ExternalInput")
    xsh = nc.dram_tensor("xsh", [SHARD, D], f32, kind="ExternalInput")
    idxs = nc.dram_tensor("idxs", [P, ICOLS], mybir.dt.int16, kind="ExternalInput")
    dstsel = nc.dram_tensor("dstsel", [P, COLS], bf16, kind="ExternalInput")
    dinvt = nc.dram_tensor("dinvt", [P, TPC], f32, kind="ExternalInput")
    Ws = [nc.dram_tensor(f"W{l}", [D, D], f32, kind="ExternalInput") for l in range(3)]
    brs = [nc.dram_tensor(f"br{l}", [P, D], f32, kind="ExternalInput") for l in range(3)]
    ysh = nc.dram_tensor("ysh", [SHARD, D], f32, kind="ExternalOutput")

    agin = [nc.dram_tensor(f"agin{l}", [SHARD, D], bf16) for l in range(2)]
    HROWS_T = NCORES * (TPC // 2) * P
    ofullA = [nc.dram_tensor(f"ofullA{l}", [HROWS_T, D], bf16,
                             addr_space="Shared") for l in range(2)]
    ofullB = [nc.dram_tensor(f"ofullB{l}", [NPAD - HROWS_T, D], bf16,
                             addr_space="Shared") for l in range(2)]
    res = [nc.dram_tensor(f"res{l}", [SHARD, D], f32) for l in range(2)]

    blocks_by_g = {}
    for b in sched["blocks"]:
        blocks_by_g.setdefault(b["g"], []).append(b)

    with tile.TileContext(nc) as tc:
        with (
            tc.tile_pool(name="const", bufs=1) as cp,
            tc.tile_pool(name="gath", bufs=22) as gp,
            tc.tile_pool(name="selp", bufs=3) as sp,
            tc.tile_pool(name="work", bufs=2) as wp,
            tc.tile_pool(name="pag", bufs=6, space="PSUM") as pag,
            tc.tile_pool(name="pout", bufs=2, space="PSUM") as pout,
        ):
            # --- constants ---
            idx_sb = cp.tile([P, ICOLS], mybir.dt.int16)
            nc.sync.dma_start(idx_sb[:], idxs.ap())
            ds_sb = cp.tile([P, COLS], bf16)
            nc.sync.dma_start(ds_sb[:], dstsel.ap())
            dv_sb = cp.tile([P, TPC], f32)
            nc.sync.dma_start(dv_sb[:], dinvt.ap())
            W_sb = []
            b_sb = []
            for l in range(3):
                t = cp.tile([D, D], f32, tag=f"W{l}")
                nc.sync.dma_start(t[:], Ws[l].ap())
                W_sb.append(t)
                t = cp.tile([P, D], f32, tag=f"br{l}")
                nc.sync.dma_start(t[:], brs[l].ap())
                b_sb.append(t)
            iota_i = cp.tile([P, P], mybir.dt.int32)
            nc.gpsimd.iota(iota_i[:], pattern=[[1, P]], base=0, channel_multiplier=0)
            iota_f = cp.tile([P, P], f32)
            nc.vector.tensor_copy(iota_f[:], iota_i[:])
            iota_b = cp.tile([P, P], bf16)
            nc.vector.tensor_copy(iota_b[:], iota_i[:])
            iotac_i = cp.tile([P, 1], mybir.dt.int32)
            nc.gpsimd.iota(iotac_i[:], pattern=[[0, 1]], base=0, channel_multiplier=1)
            iotac_f = cp.tile([P, 1], f32)
            nc.vector.tensor_copy(iotac_f[:], iotac_i[:])
            eye2 = cp.tile([P, P], bf16)
            nc.vector.tensor_scalar(
                out=eye2[:], in0=iota_f[:],
                scalar1=iotac_f[:], scalar2=2.0,
                op0=mybir.AluOpType.is_equal, op1=mybir.AluOpType.mult)

            qrr = [0]
            pending_agb = [None]

            def emit_agb():
                if pending_agb[0] is None:
                    return
                l = pending_agb[0]
                pending_agb[0] = None
                with nc.named_scope(f"ag{l}"):
                    nc.gpsimd.collective_compute(
                        "AllGather",
                        mybir.AluOpType.bypass,
                        replica_groups=[list(range(NCORES))],
                        ins=[agin[l].ap()[(TPC // 2) * P:, :]],
                        outs=[ofullB[l].ap()],
                    )

            for i, b in enumerate(sched["blocks"]):
                b["bidx"] = i
            gbank = {}

            def emit_gens(b):
                """Issue this block's dma_gather calls; matmuls attach later."""
                r = b["r"]
                gts = []
                for call in b["calls"]:
                    nch = call["nch"]
                    gt = gp.tile([P, nch, P], bf16, tag="gath", name="gt")
                    nc.gpsimd.dma_gather(
                        out_ap=gt[:],
                        in_ap=gsrcs[r],
                        idxs_ap=idx_sb[:, call["c0"] * 8:
                                       (call["c0"] + nch) * 8],
                        num_idxs=nch * P,
                        num_idxs_reg=nch * P,
                        elem_size=D,
                        elem_step=D,
                        queue_num=qrr[0] % 4,
                    )
                    qrr[0] += 1
                    gts.append(gt)
                return gts

            for layer in range(nlayers):
                if layer == 0:
                    gsrcs = [tfull0.ap()[r * RSZ:(r + 1) * RSZ, :]
                             for r in range(NR)]
                else:
                    gsrcs = [
                        ofullA[layer - 1].ap()[:RSZ, :],
                        ofullA[layer - 1].ap()[RSZ:, :],
                        ofullB[layer - 1].ap()[:RSZ, :],
                        ofullB[layer - 1].ap()[RSZ:, :],
                    ]
                selfsrc = [tsh0, agin[0], agin[1]][layer]
                resid = [xsh, res[0], res[1]][layer]
                res_next = [res[0], res[1], None][layer]
                out_t = ysh if layer == nlayers - 1 else None
                HT = TPC // 2
                HROWS = NCORES * HT * P
                with nc.named_scope(f"layer{layer}"):
                    for g in range(NG):
                        T = GS[g]
                        t0 = int(t0_of_g[g])
                        psums = [pag.tile([P, P], f32, tag="agg",
                                         name=f"ps_l{layer}_g{g}_{i}")
                                 for i in range(T)]
                        selfg = wp.tile([P, T, P], bf16, tag="selfg",
                                        name="selfg")
                        nc.sync.dma_start(
                            selfg[:],
                            selfsrc.ap()[t0 * P:(t0 + T) * P, :]
                            .rearrange("(c p) d -> p c d", p=P))
                        for tl in range(T):
                            nc.tensor.matmul(
                                out=psums[tl][:], lhsT=selfg[:, tl, :],
                                rhs=eye2[:], start=True, stop=False)
                        for b in blocks_by_g[g]:
                            r = b["r"]
                            # the previous layer's second-half AllGather is
                            # emitted between ranges 1 and 2 of group 0; the
                            # next groups' range-0/1 gathers (which depend
                            # only on the long-done first-half AllGather) are
                            # issued right after it, keeping the Pool engine
                            # generating descriptors through the collective's
                            # mesh window instead of stalling on range 2
                            if g == 0 and r == 2:
                                emit_agb()
                                if layer > 0:
                                    for gg in range(1, 4):
                                        for bb in blocks_by_g[gg][:2]:
                                            gbank[bb["bidx"]] = \
                                                emit_gens(bb)
                            nbc = b["ncols"]
                            selb = sp.tile([P, nbc, P], bf16, tag="sel",
                                           name="selb")
                            nc.vector.tensor_tensor(
                                out=selb[:],
                                in0=iota_b[:].rearrange(
                                    "p (c m) -> p c m", c=1
                                ).to_broadcast([P, nbc, P]),
                                in1=ds_sb[:, b["col0"]:b["col0"] + nbc]
                                .rearrange("p (c m) -> p c m", m=1)
                                .to_broadcast([P, nbc, P]),
                                op=mybir.AluOpType.is_equal,
                            )
                            gts = gbank.pop(b["bidx"], None)
                            if gts is None:
                                gts = emit_gens(b)
                            for ci, call in enumerate(b["calls"]):
                                gt = gts[ci]
                                boff = call["col0"] - b["col0"]
                                for (co, cc, tl, stop) in call["mms"]:
                                    nc.tensor.matmul(
                                        out=psums[tl][:],
                                        lhsT=gt[:, cc, :],
                                        rhs=selb[:, boff + co, :],
                                        start=False, stop=stop)
                        # ---- epilogue for this group (batched) ----
                        old = wp.tile([P, T, P], f32, tag="old", name="old")
                        nc.sync.dma_start(
                            old[:], resid.ap()[t0 * P:(t0 + T) * P, :]
                            .rearrange("(c p) d -> p c d", p=P))
                        if layer == 1:
                            xres = wp.tile([P, T, P], f32, tag="xres",
                                           name="xres")
                            nc.sync.dma_start(
                                xres[:], xsh.ap()[t0 * P:(t0 + T) * P, :]
                                .rearrange("(c p) d -> p c d", p=P))
                        aggT = wp.tile([P, T, P], f32, tag="aggT", name="aggT")
                        outn = wp.tile([P, T, P], f32, tag="outn", name="outn")
                        scl = (wp.tile([P, T, P], bf16, tag="scl", name="scl")
                               if out_t is None else None)
                        for tl in range(T):
                            t = t0 + tl
                            nc.scalar.activation(
                                out=aggT[:, tl, :], in_=psums[tl][:],
                                func=mybir.ActivationFunctionType.Copy)
                            pso = pout.tile([P, P], f32, tag="out", name="pso")
                            nc.tensor.matmul(out=pso[:], lhsT=aggT[:, tl, :],
                                             rhs=W_sb[layer][:],
                                             start=True, stop=True)
                            nc.scalar.activation(
                                out=outn[:, tl, :], in_=pso[:],
                                func=mybir.ActivationFunctionType.Copy,
                                scale=dv_sb[:, t:t + 1])
                            nc.vector.tensor_add(outn[:, tl, :],
                                                 outn[:, tl, :],
                                                 old[:, tl, :])
                            nc.vector.tensor_add(outn[:, tl, :],
                                                 outn[:, tl, :],
                                                 b_sb[layer][:])
                            if scl is not None:
                                nc.scalar.activation(
                                    out=scl[:, tl, :], in_=outn[:, tl, :],
                                    func=mybir.ActivationFunctionType.Copy,
                                    scale=dv_sb[:, t:t + 1])
                            if layer == 1:
                                nc.vector.tensor_add(outn[:, tl, :],
                                                     outn[:, tl, :],
                                                     xres[:, tl, :])
                        if out_t is not None:
                            nc.sync.dma_start(
                                out_t.ap()[t0 * P:(t0 + T) * P, :]
                                .rearrange("(c p) d -> p c d", p=P), outn[:])
                        else:
                            nc.sync.dma_start(
                                res_next.ap()[t0 * P:(t0 + T) * P, :]
                                .rearrange("(c p) d -> p c d", p=P), outn[:])
                            nc.sync.dma_start(
                                agin[layer].ap()[t0 * P:(t0 + T) * P, :]
                                .rearrange("(c p) d -> p c d", p=P), scl[:])
                        # the first half of the shard is complete after group
                        # 8; exchange it a few groups later so the epilogue
                        # pipeline has drained (no Pool-sequencer stall)
                        if layer < nlayers - 1 and g == 11:
                            with nc.named_scope(f"ag{layer}a"):
                                nc.gpsimd.collective_compute(
                                    "AllGather",
                                    mybir.AluOpType.bypass,
                                    replica_groups=[list(range(NCORES))],
                                    ins=[agin[layer].ap()[:HT * P, :]],
                                    outs=[ofullA[layer].ap()],
                                )
                if layer < nlayers - 1:
                    pending_agb[0] = layer
    nc.compile()
    _NC_CACHE[key] = nc
    return nc


def _make_in_maps(prep, x, W0, b0, W1, b1, W2, b2, cfg):
    bf16 = prep["bf16"]
    x = np.asarray(x, np.float32)
    x_pad = np.zeros((cfg.NPAD, D), np.float32)
    x_pad[prep["pid"][:cfg.N]] = x
    t0 = (x_pad * prep["dinv_pad"][:, None]).astype(bf16)
    # full table in half-major row order
    t0_table = np.empty_like(t0)
    t0_table[prep["trow_of"]] = t0

    bl = [np.broadcast_to(np.asarray(b, np.float32), (P, D)).copy()
          for b in (b0, b1, b2)]
    Wl = [np.ascontiguousarray(np.asarray(w, np.float32)) for w in (W0, W1, W2)]
    maps = []
    for k in range(NCORES):
        sl = slice(k * cfg.SHARD, (k + 1) * cfg.SHARD)
        m = {
            "tfull0": t0_table,
            "tsh0": np.ascontiguousarray(t0[sl]),
            "xsh": np.ascontiguousarray(x_pad[sl]),
            "idxs": np.ascontiguousarray(prep["idxs16"][k]),
            "dstsel": np.ascontiguousarray(prep["dstsel"][k]),
            "dinvt": np.ascontiguousarray(prep["dinv_tiles"][k]),
        }
        for l in range(3):
            m[f"W{l}"] = Wl[l]
            m[f"br{l}"] = bl[l]
        maps.append(m)
    return maps


_PREP_CACHE = {}


def _run(x, edge_index, W0, b0, W1, b1, W2, b2, cfg, trace=False, nlayers=3,
         trace_cores=None):
    from concourse.bass_utils import run_bass_kernel_spmd

    edge_index = np.asarray(edge_index)
    key = (edge_index.tobytes()[:4096], edge_index.shape,
           int(edge_index[:, ::997].sum()))
    if key in _PREP_CACHE:
        prep = _PREP_CACHE[key]
    else:
        prep = _host_prep(edge_index, cfg)
        _PREP_CACHE.clear()
        _PREP_CACHE[key] = prep

    nc = _build_nc(cfg, prep["sched"], nlayers=nlayers)
    in_maps = _make_in_maps(prep, x, W0, b0, W1, b1, W2, b2, cfg)
    res = run_bass_kernel_spmd(
        nc, in_maps, core_ids=list(range(NCORES)), trace=trace,
        trace_cores=trace_cores)
    ysh = np.concatenate([res.results[k]["ysh"] for k in range(NCORES)], axis=0)
    y = ysh[prep["pid"][:cfg.N]]
    return y, res


def kernel(x, edge_index, W0, b0, W1, b1, W2, b2):
    y, _ = _run(x, edge_index, W0, b0, W1, b1, W2, b2, CFG, trace=False)
    return y
